# revision 3
# baseline (speedup 1.0000x reference)
"""MLA (multi-head latent attention) Trainium2 kernel, 8-core SPMD.

Sharding: batch (2) x head-groups (4 groups of 4 heads) -> 8 cores.
Each core computes a partial output (its 4 heads' contribution through wo);
host sums the 4 partials per batch.

All matmuls run in float32r (fp32 with 11-bit mantissa, 1 cycle/row on PE).
Weights and x are pre-rounded to f32r on the host (bitwise-exact match with
the hardware rounding); intermediate activations are rounded by the
PSUM->SBUF eviction copies.

Layout convention: activations are stored transposed ([feature, seq]) so
that every matmul consumes natural-layout DRAM weights, except v which is
kept as [seq, dim] for the PV matmul. Attention computes scoresT [t, s]
blocks; softmax uses exp without max-subtraction (scores are bounded by
construction) so the denominator is a ones-matmul partition reduction and
normalization happens after the PV matmul.
"""
import os
import sys
import numpy as np
from contextlib import ExitStack

sys.path.insert(0, "/opt/trn_rl_repo")

import concourse.bass as bass
import concourse.mybir as mybir
import concourse.tile as tile
from concourse.bass_utils import run_bass_kernel_spmd

F32 = mybir.dt.float32
F32R = mybir.dt.float32r
AF = mybir.ActivationFunctionType

B, S, D = 2, 2048, 2048
H, NOPE, RD, VD = 16, 128, 64, 128
QR, KVR, EPS = 1536, 512, 1e-6
HPC = 4          # heads per core
SCALE = 1.0 / np.sqrt(NOPE + RD)
NEG = -1e30

_CACHED = {}


def _round_f32r(x):
    x = np.ascontiguousarray(x, dtype=np.float32)
    u = x.view(np.uint32)
    low = u & np.uint32(0xFFF)
    half = np.uint32(1 << 11)
    u_trunc = u & np.uint32(0xFFFFF000)
    lsb = (u >> np.uint32(12)) & np.uint32(1)
    up = (low > half) | ((low == half) & (lsb == 1))
    return (u_trunc + (up.astype(np.uint32) << np.uint32(12))).view(np.float32)


def _split_waits(nc, max_waits=1):
    # This walrus build only supports one sem-wait per instruction; move
    # extra waits onto preceding same-engine NOPs.
    ctr = 0
    for f in nc.m.functions:
        for bb in f.blocks:
            insts = list(bb.instructions)
            out = []
            changed = False
            for inst in insts:
                si = inst.sync_info
                waits = list(si.on_wait) if si is not None else []
                if len(waits) > max_waits:
                    changed = True
                    head, rest = waits[:max_waits], waits[max_waits:]
                    while rest:
                        ctr += 1
                        nop = mybir.InstNoOp(name=f"WS-{ctr}")
                        nop.engine = inst.engine
                        nop.sync_info = mybir.SyncInfo(on_wait=head, on_update=[])
                        out.append(nop)
                        head, rest = rest[:max_waits], rest[max_waits:]
                    inst.sync_info = mybir.SyncInfo(
                        on_wait=head, on_update=list(si.on_update))
                out.append(inst)
            if changed:
                bb.instructions = out


def _build_nc():
    nc = bass.Bass("TRN2", target_bir_lowering=False, debug=False, num_devices=8)

    def din(name, shape, dt=F32R):
        return nc.dram_tensor(name, list(shape), dt, kind="ExternalInput").ap()

    xT = din("xT", [D, S])                      # x[b].T
    wqd = din("wqd", [D, QR])
    wqu = din("wqu", [QR, HPC * NOPE])          # *SCALE*gamma folded
    wqr = din("wqr", [QR, HPC * RD])            # *SCALE folded
    wkvd = din("wkvd", [D, KVR])
    wkuk = din("wkuk", [KVR, HPC * NOPE])       # kv gamma folded
    wkuv = din("wkuv", [KVR, HPC * VD])
    wkr = din("wkr", [D, RD])
    wo = din("wo", [HPC * VD, D])
    cosq = din("cosq", [128, S], F32)           # 2-head-stacked rope tables
    sinq = din("sinq", [128, S], F32)
    cosk = din("cosk", [RD, S], F32)
    sink = din("sink", [RD, S], F32)
    perm128 = din("perm128", [128, 128])
    perm64 = din("perm64", [RD, RD])
    ones_col = din("ones_col", [128, 1])
    ones_row = din("ones_row", [1, 128])
    eps_col = din("eps_col", [128, 1], F32)
    trimask = din("trimask", [128, 128], F32)   # [t,s]: 0 if s>=t else NEG

    outT = nc.dram_tensor("outT", [D, S], F32, kind="ExternalOutput").ap()

    # rearranged DRAM views: [part, kchunk, cols]
    xT_r = xT.rearrange("(k p) s -> p k s", p=128)
    wqd_r = wqd.rearrange("(k p) q -> p k q", p=128)
    wqu_r = wqu.rearrange("(k p) q -> p k q", p=128)
    wqr_r = wqr.rearrange("(k p) q -> p k q", p=128)
    wkvd_r = wkvd.rearrange("(k p) q -> p k q", p=128)
    wkuk_r = wkuk.rearrange("(k p) q -> p k q", p=128)
    wkuv_r = wkuv.rearrange("(k p) q -> p k q", p=128)
    wkr_r = wkr.rearrange("(k p) q -> p k q", p=128)
    wo_r = wo.rearrange("(k p) q -> p k q", p=128)

    NQC = QR // 128     # 12
    NKC = KVR // 128    # 4

    with tile.TileContext(nc) as tc, ExitStack() as top:
        dpool = top.enter_context(tc.tile_pool(name="dram", bufs=1, space="DRAM"))
        qn_sp = dpool.tile([HPC, 128, S], F32R, tag="qns", name="qn_sp")
        qr_sp = dpool.tile([2, 128, S], F32R, tag="qrs", name="qr_sp")

        consts = top.enter_context(tc.tile_pool(name="consts", bufs=1))
        ones_c = consts.tile([128, 1], F32R, tag="oc", name="ones_c")
        nc.sync.dma_start(ones_c[:], ones_col[:])
        ones_r = consts.tile([1, 128], F32R, tag="orr", name="ones_r")
        nc.sync.dma_start(ones_r[:], ones_row[:])
        eps_c = consts.tile([128, 1], F32, tag="eps", name="eps_c")
        nc.sync.dma_start(eps_c[:], eps_col[:])

        # ---------------- Sweep 1: Q path ----------------
        with ExitStack() as s1:
            p_x = s1.enter_context(tc.tile_pool(name="s1x", bufs=1))
            p_w = s1.enter_context(tc.tile_pool(name="s1w", bufs=1))
            p_t = s1.enter_context(tc.tile_pool(name="s1t", bufs=1))
            psum = s1.enter_context(tc.tile_pool(name="s1p", bufs=1, space="PSUM"))

            wqu_sb = p_w.tile([128, NQC, HPC * NOPE], F32R, tag="wqu",
                              name="wqu_sb")
            nc.sync.dma_start(wqu_sb[:], wqu_r[:])
            wqr_sb = p_w.tile([128, NQC, HPC * RD], F32R, tag="wqr",
                              name="wqr_sb")
            nc.sync.dma_start(wqr_sb[:], wqr_r[:])
            perm_sb = p_w.tile([128, 128], F32R, tag="pm", name="perm_sb")
            nc.sync.dma_start(perm_sb[:], perm128[:])

            for n in range(4):
                c0 = n * 512
                sl = slice(c0, c0 + 512)
                xt = p_x.tile([128, 16, 512], F32R, tag="xt", name=f"xt{n}")
                nc.sync.dma_start(xt[:], xT_r[:, :, sl])
                qc = p_x.tile([128, NQC, 512], F32R, tag="qc", name=f"qc{n}")
                cq = p_w.tile([128, 512], F32, tag="cq", bufs=2, name=f"cq{n}")
                nc.sync.dma_start(cq[:], cosq[:, sl])
                sqt = p_w.tile([128, 512], F32, tag="sqt", bufs=2, name=f"sqt{n}")
                nc.sync.dma_start(sqt[:], sinq[:, sl])
                ss = psum.tile([1, 512], F32, tag="ss", name=f"ss{n}")
                for m in range(NQC):
                    wm = p_w.tile([128, 16, 128], F32R, tag="wm", bufs=2,
                                  name=f"wqd{n}_{m}")
                    nc.sync.dma_start(wm[:], wqd_r[:, :, m * 128:(m + 1) * 128])
                    ps = psum.tile([128, 512], F32, tag="mm", bufs=2,
                                   name=f"qm{n}_{m}")
                    for k in range(16):
                        nc.tensor.matmul(ps[:], wm[:, k, :], xt[:, k, :],
                                         start=(k == 0), stop=(k == 15),
                                         skip_group_check=True)
                    nc.vector.tensor_copy(qc[:, m, :], ps[:])
                    sq = p_t.tile([128, 512], F32R, tag="sq", bufs=2,
                                  name=f"sq{n}_{m}")
                    nc.scalar.activation(sq[:], ps[:], AF.Square)
                    nc.tensor.matmul(ss[:], ones_c[:], sq[:],
                                     start=(m == 0), stop=(m == NQC - 1),
                                     skip_group_check=True)
                rms = p_t.tile([1, 512], F32R, tag="rms", name=f"rms{n}")
                nc.scalar.activation(rms[:], ss[:], AF.Sqrt,
                                     bias=eps_c[0:1, :], scale=1.0 / QR)
                pbc = psum.tile([128, 512], F32, tag="bc", name=f"bc{n}")
                nc.tensor.matmul(pbc[:], ones_r[:], rms[:], start=True,
                                 stop=True, skip_group_check=True)
                bq = p_t.tile([128, 512], F32, tag="bq", name=f"bq{n}")
                nc.vector.reciprocal(bq[:], pbc[:])
                # q_nope up-proj (normalized at evict)
                for mh in range(HPC):
                    ps = psum.tile([128, 512], F32, tag="mm", bufs=2,
                                   name=f"qu{n}_{mh}")
                    for k in range(NQC):
                        nc.tensor.matmul(ps[:], wqu_sb[:, k, mh * 128:(mh + 1) * 128],
                                         qc[:, k, :],
                                         start=(k == 0), stop=(k == NQC - 1),
                                         skip_group_check=True)
                    nt = p_t.tile([128, 512], F32R, tag="nt", bufs=2,
                                  name=f"nt{n}_{mh}")
                    nc.vector.tensor_mul(nt[:], ps[:], bq[:])
                    nc.sync.dma_start(qn_sp[mh, :, sl], nt[:])
                # q_rope up-proj + rope + normalize
                for mr in range(2):
                    ps = psum.tile([128, 512], F32, tag="mm", bufs=2,
                                   name=f"qrp{n}_{mr}")
                    for k in range(NQC):
                        nc.tensor.matmul(ps[:], wqr_sb[:, k, mr * 128:(mr + 1) * 128],
                                         qc[:, k, :],
                                         start=(k == 0), stop=(k == NQC - 1),
                                         skip_group_check=True)
                    raw = p_t.tile([128, 512], F32R, tag="rraw", bufs=2,
                                   name=f"rr{n}_{mr}")
                    nc.scalar.copy(raw[:], ps[:])
                    psw = psum.tile([128, 512], F32, tag="mm", bufs=2,
                                    name=f"sw{n}_{mr}")
                    nc.tensor.matmul(psw[:], perm_sb[:], raw[:], start=True,
                                     stop=True, skip_group_check=True)
                    t1 = p_t.tile([128, 512], F32, tag="rt1", bufs=2,
                                  name=f"t1{n}_{mr}")
                    nc.vector.tensor_mul(t1[:], raw[:].bitcast(F32), cq[:])
                    t2 = p_t.tile([128, 512], F32, tag="rt2", bufs=2,
                                  name=f"t2{n}_{mr}")
                    nc.vector.tensor_mul(t2[:], psw[:], sqt[:])
                    t3 = p_t.tile([128, 512], F32, tag="rt3", bufs=2,
                                  name=f"t3{n}_{mr}")
                    nc.vector.tensor_add(t3[:], t1[:], t2[:])
                    rp = p_t.tile([128, 512], F32R, tag="rp", bufs=2,
                                  name=f"rp{n}_{mr}")
                    nc.vector.tensor_mul(rp[:], t3[:], bq[:])
                    nc.sync.dma_start(qr_sp[mr, :, sl], rp[:])

        # pools spanning sweeps 2a..4
        mid = top.enter_context(ExitStack())
        p_kr = mid.enter_context(tc.tile_pool(name="pkr", bufs=1))
        krope = p_kr.tile([RD, S], F32R, tag="kr", name="krope")
        p_kvn = mid.enter_context(tc.tile_pool(name="pkvn", bufs=1))
        kvn = p_kvn.tile([128, NKC, S], F32R, tag="kvn", name="kvn")

        # ---------------- Sweep 2a: kv_c + k_rope ----------------
        with ExitStack() as s2:
            p_x = s2.enter_context(tc.tile_pool(name="s2x", bufs=1))
            p_w = s2.enter_context(tc.tile_pool(name="s2w", bufs=1))
            p_t = s2.enter_context(tc.tile_pool(name="s2t", bufs=1))
            psum = s2.enter_context(tc.tile_pool(name="s2p", bufs=1, space="PSUM"))

            wkvd_sb = p_w.tile([128, 16, KVR], F32R, tag="wkvd", name="wkvd_sb")
            nc.sync.dma_start(wkvd_sb[:], wkvd_r[:])
            wkr_sb = p_w.tile([128, 16, RD], F32R, tag="wkr", name="wkr_sb")
            nc.sync.dma_start(wkr_sb[:], wkr_r[:])
            perm64_sb = p_w.tile([RD, RD], F32R, tag="pm64", name="perm64_sb")
            nc.sync.dma_start(perm64_sb[:], perm64[:])

            for n in range(4):
                c0 = n * 512
                sl = slice(c0, c0 + 512)
                xt = p_x.tile([128, 16, 512], F32R, tag="xt2", name=f"x2_{n}")
                nc.sync.dma_start(xt[:], xT_r[:, :, sl])
                ck = p_w.tile([RD, 512], F32, tag="ck", bufs=2, name=f"ck{n}")
                nc.sync.dma_start(ck[:], cosk[:, sl])
                sk = p_w.tile([RD, 512], F32, tag="sk", bufs=2, name=f"sk{n}")
                nc.sync.dma_start(sk[:], sink[:, sl])
                kv_raw = p_t.tile([128, NKC, 512], F32R, tag="kvraw",
                                  name=f"kvraw{n}")
                ss = psum.tile([1, 512], F32, tag="ss2", name=f"ss2_{n}")
                for m in range(NKC):
                    ps = psum.tile([128, 512], F32, tag="mm2", bufs=2,
                                   name=f"kv{n}_{m}")
                    for k in range(16):
                        nc.tensor.matmul(ps[:], wkvd_sb[:, k, m * 128:(m + 1) * 128],
                                         xt[:, k, :],
                                         start=(k == 0), stop=(k == 15),
                                         skip_group_check=True)
                    nc.vector.tensor_copy(kv_raw[:, m, :], ps[:])
                    sq = p_t.tile([128, 512], F32R, tag="sq2", bufs=2,
                                  name=f"sq2_{n}_{m}")
                    nc.scalar.activation(sq[:], ps[:], AF.Square)
                    nc.tensor.matmul(ss[:], ones_c[:], sq[:],
                                     start=(m == 0), stop=(m == NKC - 1),
                                     skip_group_check=True)
                rms = p_t.tile([1, 512], F32R, tag="rms2", name=f"rms2_{n}")
                nc.scalar.activation(rms[:], ss[:], AF.Sqrt,
                                     bias=eps_c[0:1, :], scale=1.0 / KVR)
                pbc = psum.tile([128, 512], F32, tag="bc2", name=f"bc2_{n}")
                nc.tensor.matmul(pbc[:], ones_r[:], rms[:], start=True,
                                 stop=True, skip_group_check=True)
                bkv = p_t.tile([128, 512], F32, tag="bkv", name=f"bkv{n}")
                nc.vector.reciprocal(bkv[:], pbc[:])
                for m in range(NKC):
                    nc.vector.tensor_mul(kvn[:, m, sl],
                                         kv_raw[:, m, :].bitcast(F32), bkv[:])
                # k_rope
                pkr_ = psum.tile([RD, 512], F32, tag="kr2", name=f"krp{n}")
                for k in range(16):
                    nc.tensor.matmul(pkr_[:], wkr_sb[:, k, :], xt[:, k, :],
                                     start=(k == 0), stop=(k == 15),
                                     skip_group_check=True)
                rawk = p_t.tile([RD, 512], F32R, tag="rawk", name=f"rawk{n}")
                nc.scalar.copy(rawk[:], pkr_[:])
                psw = psum.tile([RD, 512], F32, tag="sw64", name=f"swk{n}")
                nc.tensor.matmul(psw[:], perm64_sb[:], rawk[:], start=True,
                                 stop=True, skip_group_check=True)
                t1 = p_t.tile([RD, 512], F32, tag="kt1", name=f"kt1_{n}")
                nc.vector.tensor_mul(t1[:], rawk[:].bitcast(F32), ck[:])
                t2 = p_t.tile([RD, 512], F32, tag="kt2", name=f"kt2_{n}")
                nc.vector.tensor_mul(t2[:], psw[:], sk[:])
                nc.vector.tensor_add(krope[:, sl], t1[:], t2[:])

        # ---------------- Sweep 2b: k_nope + v ----------------
        p_kv2 = mid.enter_context(tc.tile_pool(name="pkv2", bufs=1))
        knope = [p_kv2.tile([128, S], F32R, tag=f"kn{h}", name=f"knope{h}")
                 for h in range(HPC)]
        v_sb = [p_kv2.tile([128, 16, 2 * VD], F32R, tag=f"vp{p}", name=f"v_sb{p}")
                for p in range(2)]
        with ExitStack() as s2b:
            p_w = s2b.enter_context(tc.tile_pool(name="s2bw", bufs=1))
            psum = s2b.enter_context(tc.tile_pool(name="s2bp", bufs=1, space="PSUM"))
            wkuk_sb = p_w.tile([128, NKC, HPC * NOPE], F32R, tag="wkuk",
                               name="wkuk_sb")
            nc.sync.dma_start(wkuk_sb[:], wkuk_r[:])
            wkuv_sb = p_w.tile([128, NKC, HPC * VD], F32R, tag="wkuv",
                               name="wkuv_sb")
            nc.sync.dma_start(wkuv_sb[:], wkuv_r[:])
            for mh in range(HPC):
                for c in range(4):
                    sl = slice(c * 512, c * 512 + 512)
                    ps = psum.tile([128, 512], F32, tag="mm2b", bufs=3,
                                   name=f"knp{mh}_{c}")
                    for k in range(NKC):
                        nc.tensor.matmul(ps[:], wkuk_sb[:, k, mh * 128:(mh + 1) * 128],
                                         kvn[:, k, sl],
                                         start=(k == 0), stop=(k == NKC - 1),
                                         skip_group_check=True)
                    if (mh + c) % 2 == 0:
                        nc.scalar.copy(knope[mh][:, sl], ps[:])
                    else:
                        nc.vector.tensor_copy(knope[mh][:, sl], ps[:])
            for t in range(16):
                for p in range(2):
                    psv = psum.tile([128, 256], F32, tag="mmv", bufs=3,
                                    name=f"v{t}_{p}")
                    for k in range(NKC):
                        nc.tensor.matmul(psv[:], kvn[:, k, t * 128:(t + 1) * 128],
                                         wkuv_sb[:, k, p * 256:(p + 1) * 256],
                                         start=(k == 0), stop=(k == NKC - 1),
                                         skip_group_check=True)
                    if (t + p) % 2 == 0:
                        nc.scalar.copy(v_sb[p][:, t, :], psv[:])
                    else:
                        nc.vector.tensor_copy(v_sb[p][:, t, :], psv[:])

        # ---------------- Sweep 3: attention ----------------
        p_oh = mid.enter_context(tc.tile_pool(name="poh", bufs=1))
        oh = [p_oh.tile([128, S], F32R, tag=f"oh{h}", name=f"oh{h}")
              for h in range(HPC)]
        with ExitStack() as s3:
            p_q = s3.enter_context(tc.tile_pool(name="s3q", bufs=1))
            p_t = s3.enter_context(tc.tile_pool(name="s3t", bufs=1))
            psum = s3.enter_context(tc.tile_pool(name="s3p", bufs=1, space="PSUM"))
            tri_sb = p_q.tile([128, 128], F32, tag="tri", name="tri_sb")
            nc.sync.dma_start(tri_sb[:], trimask[:])

            for h in range(HPC):
                for c in range(4):
                    s0 = c * 512
                    qn = p_q.tile([128, 512], F32R, tag="qn", bufs=2,
                                  name=f"qn{h}_{c}")
                    nc.sync.dma_start(qn[:], qn_sp[h, :, s0:s0 + 512])
                    qr_ = p_q.tile([RD, 512], F32R, tag="qr", bufs=2,
                                   name=f"qr{h}_{c}")
                    nc.sync.dma_start(
                        qr_[:], qr_sp[h // 2, (h % 2) * RD:(h % 2 + 1) * RD,
                                      s0:s0 + 512])
                    po = psum.tile([128, 512], F32, tag="po", bufs=2,
                                   name=f"po{h}_{c}")
                    pd = psum.tile([1, 512], F32, tag="pd", bufs=2,
                                   name=f"pd{h}_{c}")
                    jmax = 4 * c + 3
                    for j in range(jmax + 1):
                        off = max(0, 128 * (j - 4 * c))
                        npx = 512 - off
                        sc = psum.tile([128, 512], F32, tag="sc", bufs=2,
                                       name=f"sc{h}_{c}_{j}")
                        nc.tensor.matmul(sc[:, off:], knope[h][:, j * 128:(j + 1) * 128],
                                         qn[:, off:], start=True, stop=False,
                                         skip_group_check=True)
                        nc.tensor.matmul(sc[:, off:], krope[:, j * 128:(j + 1) * 128],
                                         qr_[:, off:], start=False, stop=True,
                                         skip_group_check=True)
                        if j >= 4 * c:
                            nc.vector.tensor_add(sc[:, off:off + 128],
                                                 sc[:, off:off + 128], tri_sb[:])
                        pr = p_t.tile([128, 512], F32R, tag="pr", bufs=3,
                                      name=f"pr{h}_{c}_{j}")
                        nc.scalar.activation(pr[:, :npx], sc[:, off:], AF.Exp)
                        nc.tensor.matmul(po[:, off:],
                                         v_sb[h // 2][:, j, (h % 2) * VD:(h % 2 + 1) * VD],
                                         pr[:, :npx],
                                         start=(j == 0), stop=(j == jmax),
                                         skip_group_check=True)
                        nc.tensor.matmul(pd[:, off:], ones_c[:], pr[:, :npx],
                                         start=(j == 0), stop=(j == jmax),
                                         skip_group_check=True)
                    den = p_t.tile([1, 512], F32R, tag="den", bufs=2,
                                   name=f"den{h}_{c}")
                    nc.scalar.copy(den[:], pd[:])
                    pbc = psum.tile([128, 512], F32, tag="abc", bufs=2,
                                    name=f"abc{h}_{c}")
                    nc.tensor.matmul(pbc[:], ones_r[:], den[:], start=True,
                                     stop=True, skip_group_check=True)
                    rcp = p_t.tile([128, 512], F32, tag="rcp", bufs=2,
                                   name=f"rcp{h}_{c}")
                    nc.vector.reciprocal(rcp[:], pbc[:])
                    nc.vector.tensor_mul(oh[h][:, s0:s0 + 512], po[:], rcp[:])

        # ---------------- Sweep 4: output projection ----------------
        with ExitStack() as s4:
            p_w = s4.enter_context(tc.tile_pool(name="s4w", bufs=1))
            p_t = s4.enter_context(tc.tile_pool(name="s4t", bufs=1))
            psum = s4.enter_context(tc.tile_pool(name="s4p", bufs=1, space="PSUM"))
            wo_sb = p_w.tile([128, HPC, D], F32R, tag="wo", name="wo_sb")
            nc.sync.dma_start(wo_sb[:], wo_r[:])
            for mo in range(16):
                for c in range(4):
                    s0 = c * 512
                    ps = psum.tile([128, 512], F32, tag="mmo", bufs=3,
                                   name=f"o{mo}_{c}")
                    for k in range(HPC):
                        nc.tensor.matmul(ps[:], wo_sb[:, k, mo * 128:(mo + 1) * 128],
                                         oh[k][:, s0:s0 + 512],
                                         start=(k == 0), stop=(k == HPC - 1),
                                         skip_group_check=True)
                    fin = p_t.tile([128, 512], F32, tag="fin", bufs=3,
                                   name=f"fin{mo}_{c}")
                    if (mo + c) % 2 == 0:
                        nc.scalar.copy(fin[:], ps[:])
                    else:
                        nc.vector.tensor_copy(fin[:], ps[:])
                    nc.sync.dma_start(outT[mo * 128:(mo + 1) * 128, s0:s0 + 512],
                                      fin[:])

    _split_waits(nc, max_waits=1)
    return nc


def _host_inputs(inputs):
    """Build the 8 per-core input maps from the full-problem inputs."""
    x = np.asarray(inputs["x"], np.float32)
    wq_down = np.asarray(inputs["wq_down"], np.float32)
    q_norm_w = np.asarray(inputs["q_norm_w"], np.float32)
    wq_up = np.asarray(inputs["wq_up"], np.float32)
    wq_rope = np.asarray(inputs["wq_rope"], np.float32)
    wkv_down = np.asarray(inputs["wkv_down"], np.float32)
    kv_norm_w = np.asarray(inputs["kv_norm_w"], np.float32)
    wkv_up = np.asarray(inputs["wkv_up"], np.float32)
    wk_rope = np.asarray(inputs["wk_rope"], np.float32)
    wo = np.asarray(inputs["wo"], np.float32)

    # rope tables ([dim, s]; o = x*C + swap(x)*Ssg)
    pos = np.arange(S, dtype=np.float64)
    invf = 1.0 / (10000.0 ** (np.arange(0, RD, 2, dtype=np.float64) / RD))
    ang = invf[:, None] * pos[None, :]              # [32, S]
    cos_p, sin_p = np.cos(ang), np.sin(ang)
    C64 = np.repeat(cos_p, 2, axis=0).astype(np.float32)          # [64, S]
    Ssg = np.empty((RD, S), np.float32)
    Ssg[0::2] = -sin_p
    Ssg[1::2] = sin_p
    cosq_t = np.vstack([C64, C64])                  # [128, S]
    sinq_t = np.vstack([Ssg, Ssg])

    perm = np.zeros((128, 128), np.float32)
    idx = np.arange(128)
    perm[idx, idx ^ 1] = 1.0

    t_i = np.arange(128)[:, None]
    s_i = np.arange(128)[None, :]
    trimask = np.where(s_i >= t_i, 0.0, NEG).astype(np.float32)

    wq_up_f = (wq_up * q_norm_w[:, None] * SCALE).astype(np.float32)
    wq_rope_f = (wq_rope * SCALE).astype(np.float32)
    # wkv_up columns: head-major [h, (nope|v)]
    wku = (wkv_up * kv_norm_w[:, None]).astype(np.float32).reshape(
        KVR, H, NOPE + VD)
    wku_k = wku[:, :, :NOPE]
    wku_v = wku[:, :, NOPE:]

    shared = {
        "wqd": _round_f32r(wq_down),
        "wkvd": _round_f32r(wkv_down),
        "wkr": _round_f32r(wk_rope),
        "cosq": cosq_t, "sinq": sinq_t,
        "cosk": C64, "sink": Ssg,
        "perm128": perm, "perm64": np.ascontiguousarray(perm[:RD, :RD]),
        "ones_col": np.ones((128, 1), np.float32),
        "eps_col": np.full((128, 1), EPS, np.float32),
        "ones_row": np.ones((1, 128), np.float32),
        "trimask": trimask,
    }
    xTs = [_round_f32r(x[b].T) for b in range(B)]
    in_maps = []
    for core in range(8):
        b, g = divmod(core, HPC)
        hs = slice(g * HPC, (g + 1) * HPC)
        m = dict(shared)
        m["xT"] = xTs[b]
        m["wqu"] = _round_f32r(
            wq_up_f.reshape(QR, H, NOPE)[:, hs].reshape(QR, HPC * NOPE))
        m["wqr"] = _round_f32r(
            wq_rope_f.reshape(QR, H, RD)[:, hs].reshape(QR, HPC * RD))
        m["wkuk"] = _round_f32r(
            np.ascontiguousarray(wku_k[:, hs]).reshape(KVR, HPC * NOPE))
        m["wkuv"] = _round_f32r(
            np.ascontiguousarray(wku_v[:, hs]).reshape(KVR, HPC * VD))
        m["wo"] = _round_f32r(
            np.ascontiguousarray(wo.reshape(H, VD, D)[hs]).reshape(HPC * VD, D))
        in_maps.append(m)
    return in_maps


LAST_EXEC_NS = None


def kernel(**inputs) -> np.ndarray:
    global LAST_EXEC_NS
    if "nc" not in _CACHED:
        _CACHED["nc"] = _build_nc()
    nc = _CACHED["nc"]
    in_maps = _host_inputs(inputs)
    trace = bool(os.environ.get("MLA_TRACE"))
    res = run_bass_kernel_spmd(nc, in_maps, core_ids=list(range(8)), trace=trace)
    LAST_EXEC_NS = res.exec_time_ns
    _CACHED["last_results"] = res
    out = np.zeros((B, S, D), np.float32)
    for core in range(8):
        b = core // HPC
        out[b] += res.results[core]["outT"].T
    return out


# revision 5
# speedup vs baseline: 1.1130x; 1.1130x over previous
"""MLA (multi-head latent attention) Trainium2 kernel, 8-core SPMD.

Sharding: batch (2) x head-groups (4 groups of 4 heads) -> 8 cores.
Each core computes a partial output (its 4 heads' contribution through wo);
host sums the 4 partials per batch.

All matmuls run in float32r (fp32 with 11-bit mantissa, 1 cycle/row on PE).
Weights and x are pre-rounded to f32r on the host (bitwise-exact match with
the hardware rounding); intermediate activations are rounded by the
PSUM->SBUF eviction copies.

Layout convention: activations are stored transposed ([feature, seq]) so
that every matmul consumes natural-layout DRAM weights, except v which is
kept as [seq, dim] for the PV matmul. Attention computes scoresT [t, s]
blocks; softmax uses exp without max-subtraction (scores are bounded by
construction) so the denominator is a ones-matmul partition reduction and
normalization happens after the PV matmul.
"""
import os
import sys
import numpy as np
from contextlib import ExitStack

sys.path.insert(0, "/opt/trn_rl_repo")

import concourse.bass as bass
import concourse.mybir as mybir
import concourse.tile as tile
from concourse.bass_utils import run_bass_kernel_spmd

F32 = mybir.dt.float32
F32R = mybir.dt.float32r
AF = mybir.ActivationFunctionType

B, S, D = 2, 2048, 2048
H, NOPE, RD, VD = 16, 128, 64, 128
QR, KVR, EPS = 1536, 512, 1e-6
HPC = 4          # heads per core
SCALE = 1.0 / np.sqrt(NOPE + RD)
NEG = -1e30

_CACHED = {}


def _round_f32r(x):
    x = np.ascontiguousarray(x, dtype=np.float32)
    u = x.view(np.uint32)
    low = u & np.uint32(0xFFF)
    half = np.uint32(1 << 11)
    u_trunc = u & np.uint32(0xFFFFF000)
    lsb = (u >> np.uint32(12)) & np.uint32(1)
    up = (low > half) | ((low == half) & (lsb == 1))
    return (u_trunc + (up.astype(np.uint32) << np.uint32(12))).view(np.float32)


def _split_waits(nc, max_waits=1):
    # This walrus build only supports one sem-wait per instruction; move
    # extra waits onto preceding same-engine NOPs.
    ctr = 0
    for f in nc.m.functions:
        for bb in f.blocks:
            insts = list(bb.instructions)
            out = []
            changed = False
            for inst in insts:
                si = inst.sync_info
                waits = list(si.on_wait) if si is not None else []
                if len(waits) > max_waits:
                    changed = True
                    head, rest = waits[:max_waits], waits[max_waits:]
                    while rest:
                        ctr += 1
                        nop = mybir.InstNoOp(name=f"WS-{ctr}")
                        nop.engine = inst.engine
                        nop.sync_info = mybir.SyncInfo(on_wait=head, on_update=[])
                        out.append(nop)
                        head, rest = rest[:max_waits], rest[max_waits:]
                    inst.sync_info = mybir.SyncInfo(
                        on_wait=head, on_update=list(si.on_update))
                out.append(inst)
            if changed:
                bb.instructions = out


def _build_nc():
    nc = bass.Bass("TRN2", target_bir_lowering=False, debug=False, num_devices=8)

    def din(name, shape, dt=F32R):
        return nc.dram_tensor(name, list(shape), dt, kind="ExternalInput").ap()

    xT = din("xT", [D, S])                      # x[b].T
    wqd = din("wqd", [D, QR])
    wqu = din("wqu", [QR, HPC * NOPE])          # *SCALE*gamma folded
    wqr = din("wqr", [QR, HPC * RD])            # *SCALE folded
    wkvd = din("wkvd", [D, KVR])
    wkuk = din("wkuk", [KVR, HPC * NOPE])       # kv gamma folded
    wkuv = din("wkuv", [KVR, HPC * VD])
    wkr = din("wkr", [D, RD])
    wo = din("wo", [HPC * VD, D])
    cosq = din("cosq", [128, S], F32)           # 2-head-stacked rope tables
    sinq = din("sinq", [128, S], F32)
    cosk = din("cosk", [RD, S], F32)
    sink = din("sink", [RD, S], F32)
    perm128 = din("perm128", [128, 128])
    perm64 = din("perm64", [RD, RD])
    ones_col = din("ones_col", [128, 1])
    ones_row = din("ones_row", [1, 128])
    eps_col = din("eps_col", [128, 1], F32)
    trimask = din("trimask", [128, 128], F32)   # [t,s]: 0 if s>=t else NEG

    outT = nc.dram_tensor("outT", [D, S], F32, kind="ExternalOutput").ap()

    # rearranged DRAM views: [part, kchunk, cols]
    xT_r = xT.rearrange("(k p) s -> p k s", p=128)
    wqd_r = wqd.rearrange("(k p) q -> p k q", p=128)
    wqu_r = wqu.rearrange("(k p) q -> p k q", p=128)
    wqr_r = wqr.rearrange("(k p) q -> p k q", p=128)
    wkvd_r = wkvd.rearrange("(k p) q -> p k q", p=128)
    wkuk_r = wkuk.rearrange("(k p) q -> p k q", p=128)
    wkuv_r = wkuv.rearrange("(k p) q -> p k q", p=128)
    wkr_r = wkr.rearrange("(k p) q -> p k q", p=128)
    wo_r = wo.rearrange("(k p) q -> p k q", p=128)

    NQC = QR // 128     # 12
    NKC = KVR // 128    # 4

    with tile.TileContext(nc) as tc, ExitStack() as top:
        dpool = top.enter_context(tc.tile_pool(name="dram", bufs=1, space="DRAM"))
        qn_sp = dpool.tile([HPC, 128, S], F32R, tag="qns", name="qn_sp")
        qr_sp = dpool.tile([2, 128, S], F32R, tag="qrs", name="qr_sp")

        consts = top.enter_context(tc.tile_pool(name="consts", bufs=1))
        ones_c = consts.tile([128, 1], F32R, tag="oc", name="ones_c")
        nc.sync.dma_start(ones_c[:], ones_col[:])
        ones_r = consts.tile([1, 128], F32R, tag="orr", name="ones_r")
        nc.sync.dma_start(ones_r[:], ones_row[:])
        eps_c = consts.tile([128, 1], F32, tag="eps", name="eps_c")
        nc.sync.dma_start(eps_c[:], eps_col[:])

        # ---------------- Sweep 1: Q path ----------------
        with ExitStack() as s1:
            p_x = s1.enter_context(tc.tile_pool(name="s1x", bufs=1))
            p_w = s1.enter_context(tc.tile_pool(name="s1w", bufs=1))
            p_t = s1.enter_context(tc.tile_pool(name="s1t", bufs=1))
            psum = s1.enter_context(tc.tile_pool(name="s1p", bufs=1, space="PSUM"))

            wqu_sb = p_w.tile([128, NQC, HPC * NOPE], F32R, tag="wqu",
                              name="wqu_sb")
            nc.sync.dma_start(wqu_sb[:], wqu_r[:])
            wqr_sb = p_w.tile([128, NQC, HPC * RD], F32R, tag="wqr",
                              name="wqr_sb")
            nc.sync.dma_start(wqr_sb[:], wqr_r[:])
            perm_sb = p_w.tile([128, 128], F32R, tag="pm", name="perm_sb")
            nc.sync.dma_start(perm_sb[:], perm128[:])

            for n in range(4):
                c0 = n * 512
                sl = slice(c0, c0 + 512)
                xt = p_x.tile([128, 16, 512], F32R, tag="xt", bufs=2, name=f"xt{n}")
                nc.sync.dma_start(xt[:], xT_r[:, :, sl])
                qc = p_x.tile([128, NQC, 512], F32R, tag="qc", bufs=2, name=f"qc{n}")
                cq = p_w.tile([128, 512], F32, tag="cq", bufs=2, name=f"cq{n}")
                nc.sync.dma_start(cq[:], cosq[:, sl])
                sqt = p_w.tile([128, 512], F32, tag="sqt", bufs=2, name=f"sqt{n}")
                nc.sync.dma_start(sqt[:], sinq[:, sl])
                ss = psum.tile([1, 512], F32, tag="ss", name=f"ss{n}")
                for m in range(NQC):
                    wm = p_w.tile([128, 16, 128], F32R, tag="wm", bufs=2,
                                  name=f"wqd{n}_{m}")
                    nc.sync.dma_start(wm[:], wqd_r[:, :, m * 128:(m + 1) * 128])
                    ps = psum.tile([128, 512], F32, tag="mm", bufs=3,
                                   name=f"qm{n}_{m}")
                    for k in range(16):
                        nc.tensor.matmul(ps[:], wm[:, k, :], xt[:, k, :],
                                         start=(k == 0), stop=(k == 15),
                                         skip_group_check=True)
                    nc.vector.tensor_copy(qc[:, m, :], ps[:])
                    sq = p_t.tile([128, 512], F32R, tag="sq", bufs=2,
                                  name=f"sq{n}_{m}")
                    nc.scalar.activation(sq[:], ps[:], AF.Square)
                    nc.tensor.matmul(ss[:], ones_c[:], sq[:],
                                     start=(m == 0), stop=(m == NQC - 1),
                                     skip_group_check=True)
                rms = p_t.tile([1, 512], F32R, tag="rms", name=f"rms{n}")
                nc.scalar.activation(rms[:], ss[:], AF.Sqrt,
                                     bias=eps_c[0:1, :], scale=1.0 / QR)
                pbc = psum.tile([128, 512], F32, tag="bc", name=f"bc{n}")
                nc.tensor.matmul(pbc[:], ones_r[:], rms[:], start=True,
                                 stop=True, skip_group_check=True)
                bq = p_t.tile([128, 512], F32, tag="bq", name=f"bq{n}")
                nc.vector.reciprocal(bq[:], pbc[:])
                # q_nope up-proj (normalized at evict)
                for mh in range(HPC):
                    ps = psum.tile([128, 512], F32, tag="mm", bufs=3,
                                   name=f"qu{n}_{mh}")
                    for k in range(NQC):
                        nc.tensor.matmul(ps[:], wqu_sb[:, k, mh * 128:(mh + 1) * 128],
                                         qc[:, k, :],
                                         start=(k == 0), stop=(k == NQC - 1),
                                         skip_group_check=True)
                    nt = p_t.tile([128, 512], F32R, tag="nt", bufs=2,
                                  name=f"nt{n}_{mh}")
                    nc.vector.tensor_mul(nt[:], ps[:], bq[:])
                    nc.sync.dma_start(qn_sp[mh, :, sl], nt[:])
                # q_rope up-proj + rope + normalize
                for mr in range(2):
                    ps = psum.tile([128, 512], F32, tag="mm", bufs=3,
                                   name=f"qrp{n}_{mr}")
                    for k in range(NQC):
                        nc.tensor.matmul(ps[:], wqr_sb[:, k, mr * 128:(mr + 1) * 128],
                                         qc[:, k, :],
                                         start=(k == 0), stop=(k == NQC - 1),
                                         skip_group_check=True)
                    raw = p_t.tile([128, 512], F32R, tag="rraw", bufs=2,
                                   name=f"rr{n}_{mr}")
                    nc.scalar.copy(raw[:], ps[:])
                    psw = psum.tile([128, 512], F32, tag="mm", bufs=3,
                                    name=f"sw{n}_{mr}")
                    nc.tensor.matmul(psw[:], perm_sb[:], raw[:], start=True,
                                     stop=True, skip_group_check=True)
                    t1 = p_t.tile([128, 512], F32, tag="rt1", bufs=2,
                                  name=f"t1{n}_{mr}")
                    nc.vector.tensor_mul(t1[:], raw[:].bitcast(F32), cq[:])
                    t2 = p_t.tile([128, 512], F32, tag="rt2", bufs=2,
                                  name=f"t2{n}_{mr}")
                    nc.vector.tensor_mul(t2[:], psw[:], sqt[:])
                    t3 = p_t.tile([128, 512], F32, tag="rt3", bufs=2,
                                  name=f"t3{n}_{mr}")
                    nc.vector.tensor_add(t3[:], t1[:], t2[:])
                    rp = p_t.tile([128, 512], F32R, tag="rp", bufs=2,
                                  name=f"rp{n}_{mr}")
                    nc.vector.tensor_mul(rp[:], t3[:], bq[:])
                    nc.sync.dma_start(qr_sp[mr, :, sl], rp[:])

        # pools spanning sweeps 2a..4
        mid = top.enter_context(ExitStack())
        p_kr = mid.enter_context(tc.tile_pool(name="pkr", bufs=1))
        krope = p_kr.tile([RD, S], F32R, tag="kr", name="krope")
        p_kvn = mid.enter_context(tc.tile_pool(name="pkvn", bufs=1))
        kvn = p_kvn.tile([128, NKC, S], F32R, tag="kvn", name="kvn")

        # ---------------- Sweep 2a: kv_c + k_rope ----------------
        with ExitStack() as s2:
            p_x = s2.enter_context(tc.tile_pool(name="s2x", bufs=1))
            p_w = s2.enter_context(tc.tile_pool(name="s2w", bufs=1))
            p_t = s2.enter_context(tc.tile_pool(name="s2t", bufs=1))
            psum = s2.enter_context(tc.tile_pool(name="s2p", bufs=1, space="PSUM"))

            wkvd_sb = p_w.tile([128, 16, KVR], F32R, tag="wkvd", name="wkvd_sb")
            nc.sync.dma_start(wkvd_sb[:], wkvd_r[:])
            wkr_sb = p_w.tile([128, 16, RD], F32R, tag="wkr", name="wkr_sb")
            nc.sync.dma_start(wkr_sb[:], wkr_r[:])
            perm64_sb = p_w.tile([RD, RD], F32R, tag="pm64", name="perm64_sb")
            nc.sync.dma_start(perm64_sb[:], perm64[:])

            for n in range(4):
                c0 = n * 512
                sl = slice(c0, c0 + 512)
                xt = p_x.tile([128, 16, 512], F32R, tag="xt2", bufs=2, name=f"x2_{n}")
                nc.sync.dma_start(xt[:], xT_r[:, :, sl])
                ck = p_w.tile([RD, 512], F32, tag="ck", bufs=2, name=f"ck{n}")
                nc.sync.dma_start(ck[:], cosk[:, sl])
                sk = p_w.tile([RD, 512], F32, tag="sk", bufs=2, name=f"sk{n}")
                nc.sync.dma_start(sk[:], sink[:, sl])
                kv_raw = p_t.tile([128, NKC, 512], F32R, tag="kvraw",
                                  name=f"kvraw{n}")
                ss = psum.tile([1, 512], F32, tag="ss2", name=f"ss2_{n}")
                for m in range(NKC):
                    ps = psum.tile([128, 512], F32, tag="mm2", bufs=3,
                                   name=f"kv{n}_{m}")
                    for k in range(16):
                        nc.tensor.matmul(ps[:], wkvd_sb[:, k, m * 128:(m + 1) * 128],
                                         xt[:, k, :],
                                         start=(k == 0), stop=(k == 15),
                                         skip_group_check=True)
                    nc.vector.tensor_copy(kv_raw[:, m, :], ps[:])
                    sq = p_t.tile([128, 512], F32R, tag="sq2", bufs=2,
                                  name=f"sq2_{n}_{m}")
                    nc.scalar.activation(sq[:], ps[:], AF.Square)
                    nc.tensor.matmul(ss[:], ones_c[:], sq[:],
                                     start=(m == 0), stop=(m == NKC - 1),
                                     skip_group_check=True)
                rms = p_t.tile([1, 512], F32R, tag="rms2", name=f"rms2_{n}")
                nc.scalar.activation(rms[:], ss[:], AF.Sqrt,
                                     bias=eps_c[0:1, :], scale=1.0 / KVR)
                pbc = psum.tile([128, 512], F32, tag="bc2", name=f"bc2_{n}")
                nc.tensor.matmul(pbc[:], ones_r[:], rms[:], start=True,
                                 stop=True, skip_group_check=True)
                bkv = p_t.tile([128, 512], F32, tag="bkv", name=f"bkv{n}")
                nc.vector.reciprocal(bkv[:], pbc[:])
                for m in range(NKC):
                    nc.vector.tensor_mul(kvn[:, m, sl],
                                         kv_raw[:, m, :].bitcast(F32), bkv[:])
                # k_rope
                pkr_ = psum.tile([RD, 512], F32, tag="kr2", name=f"krp{n}")
                for k in range(16):
                    nc.tensor.matmul(pkr_[:], wkr_sb[:, k, :], xt[:, k, :],
                                     start=(k == 0), stop=(k == 15),
                                     skip_group_check=True)
                rawk = p_t.tile([RD, 512], F32R, tag="rawk", name=f"rawk{n}")
                nc.scalar.copy(rawk[:], pkr_[:])
                psw = psum.tile([RD, 512], F32, tag="sw64", name=f"swk{n}")
                nc.tensor.matmul(psw[:], perm64_sb[:], rawk[:], start=True,
                                 stop=True, skip_group_check=True)
                t1 = p_t.tile([RD, 512], F32, tag="kt1", name=f"kt1_{n}")
                nc.vector.tensor_mul(t1[:], rawk[:].bitcast(F32), ck[:])
                t2 = p_t.tile([RD, 512], F32, tag="kt2", name=f"kt2_{n}")
                nc.vector.tensor_mul(t2[:], psw[:], sk[:])
                nc.vector.tensor_add(krope[:, sl], t1[:], t2[:])

        # ---------------- Sweep 2b: k_nope + v ----------------
        p_kv2 = mid.enter_context(tc.tile_pool(name="pkv2", bufs=1))
        knope = [p_kv2.tile([128, S], F32R, tag=f"kn{h}", name=f"knope{h}")
                 for h in range(HPC)]
        v_sb = [p_kv2.tile([128, 16, 2 * VD], F32R, tag=f"vp{p}", name=f"v_sb{p}")
                for p in range(2)]
        with ExitStack() as s2b:
            p_w = s2b.enter_context(tc.tile_pool(name="s2bw", bufs=1))
            psum = s2b.enter_context(tc.tile_pool(name="s2bp", bufs=1, space="PSUM"))
            wkuk_sb = p_w.tile([128, NKC, HPC * NOPE], F32R, tag="wkuk",
                               name="wkuk_sb")
            nc.sync.dma_start(wkuk_sb[:], wkuk_r[:])
            wkuv_sb = p_w.tile([128, NKC, HPC * VD], F32R, tag="wkuv",
                               name="wkuv_sb")
            nc.sync.dma_start(wkuv_sb[:], wkuv_r[:])
            for mh in range(HPC):
                for c in range(4):
                    sl = slice(c * 512, c * 512 + 512)
                    ps = psum.tile([128, 512], F32, tag="mm2b", bufs=3,
                                   name=f"knp{mh}_{c}")
                    for k in range(NKC):
                        nc.tensor.matmul(ps[:], wkuk_sb[:, k, mh * 128:(mh + 1) * 128],
                                         kvn[:, k, sl],
                                         start=(k == 0), stop=(k == NKC - 1),
                                         skip_group_check=True)
                    if (mh + c) % 2 == 0:
                        nc.scalar.copy(knope[mh][:, sl], ps[:])
                    else:
                        nc.vector.tensor_copy(knope[mh][:, sl], ps[:])
            for t in range(16):
                for p in range(2):
                    psv = psum.tile([128, 256], F32, tag="mmv", bufs=3,
                                    name=f"v{t}_{p}")
                    for k in range(NKC):
                        nc.tensor.matmul(psv[:], kvn[:, k, t * 128:(t + 1) * 128],
                                         wkuv_sb[:, k, p * 256:(p + 1) * 256],
                                         start=(k == 0), stop=(k == NKC - 1),
                                         skip_group_check=True)
                    if (t + p) % 2 == 0:
                        nc.scalar.copy(v_sb[p][:, t, :], psv[:])
                    else:
                        nc.vector.tensor_copy(v_sb[p][:, t, :], psv[:])

        # ---------------- Sweep 3: attention ----------------
        p_wo = mid.enter_context(tc.tile_pool(name="pwo", bufs=1))
        wo_sb = p_wo.tile([128, HPC, D], F32R, tag="wo", name="wo_sb")
        nc.sync.dma_start(wo_sb[:], wo_r[:])
        p_oh = mid.enter_context(tc.tile_pool(name="poh", bufs=1))
        oh = [p_oh.tile([128, S], F32R, tag=f"oh{h}", name=f"oh{h}")
              for h in range(HPC)]
        with ExitStack() as s3:
            p_q = s3.enter_context(tc.tile_pool(name="s3q", bufs=1))
            p_t = s3.enter_context(tc.tile_pool(name="s3t", bufs=1))
            psum = s3.enter_context(tc.tile_pool(name="s3p", bufs=1, space="PSUM"))
            tri_sb = p_q.tile([128, 128], F32, tag="tri", name="tri_sb")
            nc.sync.dma_start(tri_sb[:], trimask[:])

            for h in range(HPC):
                for c in range(4):
                    s0 = c * 512
                    qn = p_q.tile([128, 512], F32R, tag="qn", bufs=2,
                                  name=f"qn{h}_{c}")
                    nc.sync.dma_start(qn[:], qn_sp[h, :, s0:s0 + 512])
                    qr_ = p_q.tile([RD, 512], F32R, tag="qr", bufs=2,
                                   name=f"qr{h}_{c}")
                    nc.sync.dma_start(
                        qr_[:], qr_sp[h // 2, (h % 2) * RD:(h % 2 + 1) * RD,
                                      s0:s0 + 512])
                    po = psum.tile([128, 512], F32, tag="po", bufs=2,
                                   name=f"po{h}_{c}")
                    pd = psum.tile([1, 512], F32, tag="pd", bufs=2,
                                   name=f"pd{h}_{c}")
                    jmax = 4 * c + 3
                    for j in range(jmax + 1):
                        off = max(0, 128 * (j - 4 * c))
                        npx = 512 - off
                        sc = psum.tile([128, 512], F32, tag="sc", bufs=3,
                                       name=f"sc{h}_{c}_{j}")
                        nc.tensor.matmul(sc[:, off:], knope[h][:, j * 128:(j + 1) * 128],
                                         qn[:, off:], start=True, stop=False,
                                         skip_group_check=True)
                        nc.tensor.matmul(sc[:, off:], krope[:, j * 128:(j + 1) * 128],
                                         qr_[:, off:], start=False, stop=True,
                                         skip_group_check=True)
                        if j >= 4 * c:
                            nc.vector.tensor_add(sc[:, off:off + 128],
                                                 sc[:, off:off + 128], tri_sb[:])
                        pr = p_t.tile([128, 512], F32R, tag="pr", bufs=4,
                                      name=f"pr{h}_{c}_{j}")
                        nc.scalar.activation(pr[:, :npx], sc[:, off:], AF.Exp)
                        nc.tensor.matmul(po[:, off:],
                                         v_sb[h // 2][:, j, (h % 2) * VD:(h % 2 + 1) * VD],
                                         pr[:, :npx],
                                         start=(j == 0), stop=(j == jmax),
                                         skip_group_check=True)
                        nc.tensor.matmul(pd[:, off:], ones_c[:], pr[:, :npx],
                                         start=(j == 0), stop=(j == jmax),
                                         skip_group_check=True)
                    den = p_t.tile([1, 512], F32R, tag="den", bufs=2,
                                   name=f"den{h}_{c}")
                    nc.scalar.copy(den[:], pd[:])
                    pbc = psum.tile([128, 512], F32, tag="abc", bufs=1,
                                    name=f"abc{h}_{c}")
                    nc.tensor.matmul(pbc[:], ones_r[:], den[:], start=True,
                                     stop=True, skip_group_check=True)
                    rcp = p_t.tile([128, 512], F32, tag="rcp", bufs=2,
                                   name=f"rcp{h}_{c}")
                    nc.vector.reciprocal(rcp[:], pbc[:])
                    nc.vector.tensor_mul(oh[h][:, s0:s0 + 512], po[:], rcp[:])

        # ---------------- Sweep 4: output projection ----------------
        with ExitStack() as s4:
            p_w = s4.enter_context(tc.tile_pool(name="s4w", bufs=1))
            p_t = s4.enter_context(tc.tile_pool(name="s4t", bufs=1))
            psum = s4.enter_context(tc.tile_pool(name="s4p", bufs=1, space="PSUM"))
            for mo in range(16):
                for c in range(4):
                    s0 = c * 512
                    ps = psum.tile([128, 512], F32, tag="mmo", bufs=3,
                                   name=f"o{mo}_{c}")
                    for k in range(HPC):
                        nc.tensor.matmul(ps[:], wo_sb[:, k, mo * 128:(mo + 1) * 128],
                                         oh[k][:, s0:s0 + 512],
                                         start=(k == 0), stop=(k == HPC - 1),
                                         skip_group_check=True)
                    fin = p_t.tile([128, 512], F32, tag="fin", bufs=3,
                                   name=f"fin{mo}_{c}")
                    if (mo + c) % 2 == 0:
                        nc.scalar.copy(fin[:], ps[:])
                    else:
                        nc.vector.tensor_copy(fin[:], ps[:])
                    nc.sync.dma_start(outT[mo * 128:(mo + 1) * 128, s0:s0 + 512],
                                      fin[:])

    _split_waits(nc, max_waits=1)
    return nc


def _host_inputs(inputs):
    """Build the 8 per-core input maps from the full-problem inputs."""
    x = np.asarray(inputs["x"], np.float32)
    wq_down = np.asarray(inputs["wq_down"], np.float32)
    q_norm_w = np.asarray(inputs["q_norm_w"], np.float32)
    wq_up = np.asarray(inputs["wq_up"], np.float32)
    wq_rope = np.asarray(inputs["wq_rope"], np.float32)
    wkv_down = np.asarray(inputs["wkv_down"], np.float32)
    kv_norm_w = np.asarray(inputs["kv_norm_w"], np.float32)
    wkv_up = np.asarray(inputs["wkv_up"], np.float32)
    wk_rope = np.asarray(inputs["wk_rope"], np.float32)
    wo = np.asarray(inputs["wo"], np.float32)

    # rope tables ([dim, s]; o = x*C + swap(x)*Ssg)
    pos = np.arange(S, dtype=np.float64)
    invf = 1.0 / (10000.0 ** (np.arange(0, RD, 2, dtype=np.float64) / RD))
    ang = invf[:, None] * pos[None, :]              # [32, S]
    cos_p, sin_p = np.cos(ang), np.sin(ang)
    C64 = np.repeat(cos_p, 2, axis=0).astype(np.float32)          # [64, S]
    Ssg = np.empty((RD, S), np.float32)
    Ssg[0::2] = -sin_p
    Ssg[1::2] = sin_p
    cosq_t = np.vstack([C64, C64])                  # [128, S]
    sinq_t = np.vstack([Ssg, Ssg])

    perm = np.zeros((128, 128), np.float32)
    idx = np.arange(128)
    perm[idx, idx ^ 1] = 1.0

    t_i = np.arange(128)[:, None]
    s_i = np.arange(128)[None, :]
    trimask = np.where(s_i >= t_i, 0.0, NEG).astype(np.float32)

    wq_up_f = (wq_up * q_norm_w[:, None] * SCALE).astype(np.float32)
    wq_rope_f = (wq_rope * SCALE).astype(np.float32)
    # wkv_up columns: head-major [h, (nope|v)]
    wku = (wkv_up * kv_norm_w[:, None]).astype(np.float32).reshape(
        KVR, H, NOPE + VD)
    wku_k = wku[:, :, :NOPE]
    wku_v = wku[:, :, NOPE:]

    shared = {
        "wqd": _round_f32r(wq_down),
        "wkvd": _round_f32r(wkv_down),
        "wkr": _round_f32r(wk_rope),
        "cosq": cosq_t, "sinq": sinq_t,
        "cosk": C64, "sink": Ssg,
        "perm128": perm, "perm64": np.ascontiguousarray(perm[:RD, :RD]),
        "ones_col": np.ones((128, 1), np.float32),
        "eps_col": np.full((128, 1), EPS, np.float32),
        "ones_row": np.ones((1, 128), np.float32),
        "trimask": trimask,
    }
    xTs = [_round_f32r(x[b].T) for b in range(B)]
    in_maps = []
    for core in range(8):
        b, g = divmod(core, HPC)
        hs = slice(g * HPC, (g + 1) * HPC)
        m = dict(shared)
        m["xT"] = xTs[b]
        m["wqu"] = _round_f32r(
            wq_up_f.reshape(QR, H, NOPE)[:, hs].reshape(QR, HPC * NOPE))
        m["wqr"] = _round_f32r(
            wq_rope_f.reshape(QR, H, RD)[:, hs].reshape(QR, HPC * RD))
        m["wkuk"] = _round_f32r(
            np.ascontiguousarray(wku_k[:, hs]).reshape(KVR, HPC * NOPE))
        m["wkuv"] = _round_f32r(
            np.ascontiguousarray(wku_v[:, hs]).reshape(KVR, HPC * VD))
        m["wo"] = _round_f32r(
            np.ascontiguousarray(wo.reshape(H, VD, D)[hs]).reshape(HPC * VD, D))
        in_maps.append(m)
    return in_maps


LAST_EXEC_NS = None


def kernel(**inputs) -> np.ndarray:
    global LAST_EXEC_NS
    if "nc" not in _CACHED:
        _CACHED["nc"] = _build_nc()
    nc = _CACHED["nc"]
    in_maps = _host_inputs(inputs)
    trace = bool(os.environ.get("MLA_TRACE"))
    res = run_bass_kernel_spmd(nc, in_maps, core_ids=list(range(8)), trace=trace)
    LAST_EXEC_NS = res.exec_time_ns
    _CACHED["last_results"] = res
    out = np.zeros((B, S, D), np.float32)
    for core in range(8):
        b = core // HPC
        out[b] += res.results[core]["outT"].T
    return out
